# revision 1
# baseline (speedup 1.0000x reference)
"""Trainium2 Bass kernel for dynamic-RNN (LSTM, return-last) B=64 T=256 D=512 H=1024.

Strategy: data-parallel over batch across 8 NeuronCores (8 rows per core), no
inter-core communication.  Per core, everything runs in a transposed "fat"
layout with gate units on the partition axis:

  phase 1: zxT[m][p][t*8+b] = (x @ Wx + b).T for the core's 8 batch rows
           (fp16 matmuls, fp32 psum), written to DRAM.
  phase 2: 256 sequential LSTM steps.  Per step, the recurrent matmul
           zh.T = Wh.T @ h.T runs as 32 gate-chunks x 8 K-chunks of
           [128,128]x[128,8] fp16 matmuls (weights stationary), rotated
           over 4 PSUM banks (double-buffered by step parity).  Elementwise
           gates/state run on ACT/DVE in fp32 on [128,64]/[128,192] tiles.
           h is stored fp16 and dumped to a DRAM history each step.
  phase 3: the recurrence runs unmasked; dynamic_rnn's copy-through
           semantics are recovered by gathering hist[seq_len[b]-1] per row
           with register-offset DMAs.

Gate order is host-reordered to [j,i,f,o] so tanh/sigmoid each apply to one
contiguous block; FORGET_BIAS is folded into b on the host.
"""

import numpy as np

B, T, D, H = 64, 256, 512, 1024
NCORES = 8
BPC = B // NCORES              # batch rows per core
GATEMAP_NEW2OLD = [1, 0, 2, 3]  # new order [j,i,f,o] -> old (i,j,f,o) indices

_cached = {}


def _colmap():
    m = np.empty(4 * H, np.int64)
    for g in range(4):
        m[g * H:(g + 1) * H] = GATEMAP_NEW2OLD[g] * H + np.arange(H)
    return m


def _build_program(n_steps=T):
    import concourse.bass as bass
    import concourse.mybir as mybir

    f16 = mybir.dt.float16
    f32 = mybir.dt.float32
    i32 = mybir.dt.int32
    AF = mybir.ActivationFunctionType

    nc = bass.Bass()

    xT_in = nc.dram_tensor("xT", [128, 8192], f16, kind="ExternalInput")
    wx_in = nc.dram_tensor("wx", [128, 16384], f16, kind="ExternalInput")
    wh_in = nc.dram_tensor("wh", [128, 32768], f16, kind="ExternalInput")
    bT_in = nc.dram_tensor("bT", [128, 32], f32, kind="ExternalInput")
    seq_in = nc.dram_tensor("seq", [1, 8], i32, kind="ExternalInput")
    y_out = nc.dram_tensor("y", [128, 64], f32, kind="ExternalOutput")

    zx_dram = nc.dram_tensor("zx_dram", [32, 128, 2048], f32)
    hist = nc.dram_tensor("hist", [n_steps, 128, 64], f16)

    import contextlib
    stack = contextlib.ExitStack()

    sb = lambda name, shape, dt: stack.enter_context(nc.sbuf_tensor(name, shape, dt))
    sem = lambda name: stack.enter_context(nc.semaphore(name))
    xT_sb = sb("xT_sb", [128, 8192], f16)
    wx_sb = sb("wx_sb", [128, 16384], f16)
    wh_sb = sb("wh_sb", [128, 32768], f16)
    bT_sb = sb("bT_sb", [128, 32], f32)
    zst0 = sb("zst0", [128, 2048], f32)
    zst1 = sb("zst1", [128, 2048], f32)
    zxr = sb("zxr", [128, 4 * 256], f32)
    z_sb = sb("z_sb", [128, 256], f32)
    sz = sb("sz", [128, 256], f32)
    cst = sb("cst", [128, 64], f32)
    t1_sb = sb("t1_sb", [128, 64], f32)
    t2_sb = sb("t2_sb", [128, 64], f32)
    th_sb = sb("th_sb", [128, 64], f32)
    h0 = sb("h0", [128, 64], f16)
    h1 = sb("h1", [128, 64], f16)
    hout16 = sb("hout16", [128, 64], f16)
    hy = sb("hy", [128, 64], f32)
    seq_sb = sb("seq_sb", [1, 8], i32)
    spacer = sb("spacer", [128, 1], f32)
    s_in = sem("s_in"); s_p1mm = sem("s_p1mm"); s_p1e = sem("s_p1e")
    s_p1d = sem("s_p1d"); s_pe = sem("s_pe"); s_zx = sem("s_zx")
    s_zxc = sem("s_zxc"); s_za = sem("s_za"); s_act1 = sem("s_act1"); s_dvec = sem("s_dvec")
    s_act2 = sem("s_act2"); s_h = sem("s_h"); s_hist = sem("s_hist")
    s_go = sem("s_go"); s_hy = sem("s_hy"); s_yo = sem("s_yo")
    with nc.Block() as block:
        pss = [stack.enter_context(nc.psum_tensor(f"ps{i}", [128, 512], f32))
               for i in range(8)]
        zst = [zst0, zst1]
        hbuf = [h0, h1]

        # 3D strided views for the canonical z layout: col = 8*(4s+bank)+b
        def bank_view(ap2d, bank):
            v = ap2d.rearrange("p (s k b) -> p s k b", k=4, b=8)
            return v[:, :, bank, :]

        # ---------------- SYNC (SP): loads, zx store/prefetch, hist dump ----
        @block.sync
        def _(sp):
            sp.dma_start(out=xT_sb[:], in_=xT_in[:]).then_inc(s_in, 16)
            sp.dma_start(out=wx_sb[:], in_=wx_in[:]).then_inc(s_in, 16)
            sp.dma_start(out=wh_sb[:], in_=wh_in[:]).then_inc(s_in, 16)
            sp.dma_start(out=bT_sb[:], in_=bT_in[:]).then_inc(s_in, 16)
            # phase 1: store zx stage slabs
            for m in range(32):
                sp.wait_ge(s_p1e, 4 * m + 4)
                sp.dma_start(out=zx_dram[m], in_=zst[m % 2][:]).then_inc(s_p1d, 16)
            # phase 2 prefetch + hist
            zx_src = zx_dram.ap().rearrange("m p q -> p m q")
            for t in range(min(4, n_steps)):
                sp.wait_ge(s_p1d, 512)
                sp.dma_start(
                    out=zxr.ap().rearrange("p (r m b) -> p r m b", m=32, b=8)[:, t % 4],
                    in_=zx_src[:, :, 8 * t:8 * t + 8],
                ).then_inc(s_zx, 16)
            for t in range(n_steps):
                sp.wait_ge(s_h, 2 * (t + 2))
                sp.dma_start(out=hist[t], in_=hbuf[t % 2][:]).then_inc(s_hist, 16)
                if t + 4 < n_steps:
                    sp.wait_ge(s_za, 2 * (t + 1))
                    sp.dma_start(
                        out=zxr.ap().rearrange("p (r m b) -> p r m b", m=32, b=8)[:, (t + 4) % 4],
                        in_=zx_src[:, :, 8 * (t + 4):8 * (t + 4) + 8],
                    ).then_inc(s_zx, 16)

        # ---------------- TENSOR (PE): phase 1 + phase 2 matmuls ------------
        @block.tensor
        def _(te):
            te.wait_ge(s_in, 32)  # xT + wx loaded
            for m in range(32):
                for n in range(4):
                    if m >= 2:
                        te.wait_ge(s_p1e, 4 * (m - 2) + n + 1)
                    bank = (m % 2) * 4 + n
                    for k in range(4):
                        te.matmul(
                            pss[bank][:, 0:512],
                            wx_sb[:, (m * 4 + k) * 128:(m * 4 + k) * 128 + 128],
                            xT_sb[:, k * 2048 + n * 512:k * 2048 + n * 512 + 512],
                            start=(k == 0), stop=(k == 3),
                        ).then_maybe_inc((s_p1mm, 1) if k == 3 else None)
            te.wait_ge(s_in, 48)   # wh loaded
            te.wait_ge(s_p1e, 128)  # all phase-1 evacs done before reusing banks
            for t in range(n_steps):
                # A-pass (K-chunks 0-3) needs only half-0 of h(t-1); runs while
                # DVE/ACT still compute half-1. Partials go to a SEPARATE closed
                # PSUM region (cols 32:64) summed with the B partials by DVE.
                te.wait_ge(s_h, 2 * (t + 1) - 1)
                if t >= 1:
                    te.wait_ge(s_za, 2 * t)  # zadds of step t-1 done (all banks)
                for half in range(2):
                    bs = 4 * half
                    for uo in range(4 * half, 4 * half + 4):
                        for g in range(4):
                            ci = g * 8 + uo
                            for ub in range(4):
                                w_off = (ci * 8 + ub) * 128
                                te.matmul(
                                    pss[bs + g][:, 32 + 8 * (uo % 4):32 + 8 * (uo % 4) + 8],
                                    wh_sb[:, w_off:w_off + 128],
                                    hbuf[(t + 1) % 2][:, 8 * ub:8 * ub + 8],
                                    start=(ub == 0), stop=(ub == 3),
                                )
                te.wait_ge(s_h, 2 * (t + 1))   # half-1 of h(t-1)
                for half in range(2):
                    bs = 4 * half
                    for uo in range(4 * half, 4 * half + 4):
                        for g in range(4):
                            ci = g * 8 + uo
                            for ub in range(4, 8):
                                w_off = (ci * 8 + ub) * 128
                                te.matmul(
                                    pss[bs + g][:, 8 * (uo % 4):8 * (uo % 4) + 8],
                                    wh_sb[:, w_off:w_off + 128],
                                    hbuf[(t + 1) % 2][:, 8 * ub:8 * ub + 8],
                                    start=(ub == 4), stop=(ub == 7),
                                ).then_maybe_inc(
                                    (s_pe, 1) if (g == 3 and ub == 7 and uo % 4 == 3) else None)

        # ---------------- VECTOR (DVE): evacs, z-adds, state ----------------
        @block.vector
        def _(v):
            v.wait_ge(s_in, 64)
            for m in range(32):
                for n in range(4):
                    v.wait_ge(s_p1mm, 4 * m + n + 1)
                    if m >= 2 and n == 0:
                        v.wait_ge(s_p1d, 16 * (m - 1))
                    bank = (m % 2) * 4 + n
                    v.tensor_scalar_add(
                        zst[m % 2][:, 512 * n:512 * n + 512],
                        pss[bank][:, 0:512],
                        bT_sb[:, m:m + 1],
                    ).then_inc(s_p1e, 1)
            # init state
            v.memset(cst[:], 0.0)
            v.memset(hbuf[1][:], 0.0).then_inc(s_h, 2)
            for t in range(n_steps):
                zo = 256 * (t % 4)

                def zadd(H, _t=t, _zo=zo):
                    v.wait_ge(s_pe, 2 * _t + H + 1)
                    if H == 0:
                        v.wait_ge(s_zx, 16 * (_t + 1))
                    for g in range(4):
                        col = g * 64 + 32 * H
                        v.tensor_add(
                            z_sb[:, col:col + 32],
                            pss[4 * H + g][:, 0:32],
                            zxr[:, _zo + col:_zo + col + 32],
                        )
                    for g in range(4):
                        col = g * 64 + 32 * H
                        v.tensor_add(
                            z_sb[:, col:col + 32],
                            z_sb[:, col:col + 32],
                            pss[4 * H + g][:, 32:64],
                        ).then_maybe_inc((s_za, 1) if g == 3 else None)

                def stateAB(H, _t=t):
                    v.wait_ge(s_act1, 2 * _t + H + 1)
                    cc = slice(32 * H, 32 * H + 32)
                    v.tensor_mul(t2_sb[:, cc], cst[:, cc], sz[:, 128 + 32 * H:128 + 32 * H + 32])
                    v.tensor_mul(t1_sb[:, cc], sz[:, 64 + 32 * H:64 + 32 * H + 32], sz[:, 32 * H:32 * H + 32])
                    v.tensor_copy(spacer[:], t2_sb[:, 32 * H:32 * H + 1])
                    v.tensor_add(cst[:, cc], t1_sb[:, cc], t2_sb[:, cc]).then_inc(s_dvec, 1)

                def hmul(H, _t=t):
                    v.wait_ge(s_act2, 2 * _t + H + 1)
                    if H == 0 and _t >= 2:
                        v.wait_ge(s_hist, 16 * (_t - 1))
                    cc = slice(32 * H, 32 * H + 32)
                    v.tensor_mul(hbuf[_t % 2][:, cc], th_sb[:, cc],
                                 sz[:, 192 + 32 * H:192 + 32 * H + 32]).then_inc(s_h, 1)

                zadd(0)
                stateAB(0)
                zadd(1)
                hmul(0)
                stateAB(1)
                hmul(1)
            # final output convert
            v.wait_ge(s_go, 144)
            v.tensor_copy(hy[:], hout16[:]).then_inc(s_hy, 1)

        # ---------------- SCALAR (ACT): gate nonlinearities -----------------
        @block.scalar
        def _(sc):
            for t in range(n_steps):
                for H in range(2):
                    sc.wait_ge(s_za, 2 * t + H + 1)
                    col = 64 + 32 * H
                    sview = bass.AP(sz, col, [[256, 128], [64, 3], [1, 32]])
                    zview = bass.AP(z_sb, col, [[256, 128], [64, 3], [1, 32]])
                    sc.activation(sview, zview, AF.Sigmoid)
                    sc.activation(sz[:, 32 * H:32 * H + 32], z_sb[:, 32 * H:32 * H + 32],
                                  AF.Tanh).then_inc(s_act1, 1)
                    sc.wait_ge(s_dvec, 2 * t + H + 1)
                    sc.activation(th_sb[:, 32 * H:32 * H + 32], cst[:, 32 * H:32 * H + 32],
                                  AF.Tanh).then_inc(s_act2, 1)

        # ---------------- GPSIMD: final gather ------------------------------
        @block.gpsimd
        def _(g):
            g.dma_start(out=seq_sb[:], in_=seq_in[:]).then_inc(s_go, 16)
            g.wait_ge(s_go, 16)
            g.wait_ge(s_hist, 16 * n_steps)
            ctx_nc = nc.allow_non_contiguous_dma(reason="8 tiny strided gather DMAs at kernel end")
            ctx_nc.__enter__()
            for b_i in range(8):
                tr = g.alloc_register(f"t{b_i}")
                g.reg_load(tr, seq_sb[0:1, b_i:b_i + 1])
                g.reg_sub(tr, tr, 1)
                g.reg_mul(tr, tr, 8192)
                g.reg_add(tr, tr, b_i)
                g.dma_start(
                    out=bass.AP(hout16, b_i, [[64, 128], [8, 8]]),
                    in_=bass.AP(hist, tr, [[64, 128], [8, 8]]),
                ).then_inc(s_go, 16)
            ctx_nc.__exit__(None, None, None)
            g.wait_ge(s_hy, 1)
            g.dma_start(out=y_out[:], in_=hy[:]).then_inc(s_yo, 16)
            g.wait_ge(s_yo, 16)

    return nc


def _host_prep(x, seq_len, W, b, core):
    rows = slice(BPC * core, BPC * (core + 1))
    cm = _cached.setdefault("colmap", _colmap())

    xk = np.asarray(x[rows], np.float16)                 # [8,256,512]
    xT = xk.transpose(2, 1, 0).reshape(4, 128, 2048)
    xT = np.ascontiguousarray(xT.transpose(1, 0, 2)).reshape(128, 8192)

    if "wx" not in _cached:
        W16 = np.asarray(W, np.float16)
        wx_full = W16[:512][:, cm]
        wh_full = W16[512:][:, cm]
        _cached["wx"] = np.ascontiguousarray(
            wx_full.reshape(4, 128, 32, 128).transpose(1, 2, 0, 3)).reshape(128, 16384)
        _cached["wh"] = np.ascontiguousarray(
            wh_full.reshape(8, 128, 32, 128).transpose(1, 2, 0, 3)).reshape(128, 32768)
        bp = np.asarray(b, np.float32)[cm].copy()
        bp[2048:3072] += 1.0
        _cached["bT"] = np.ascontiguousarray(bp.reshape(32, 128).T)

    seq = np.asarray(seq_len[rows], np.int32).reshape(1, 8)
    return {
        "xT": xT,
        "wx": _cached["wx"],
        "wh": _cached["wh"],
        "bT": _cached["bT"],
        "seq": seq,
    }


def kernel(x, seq_len, W, b):
    from concourse.bass_utils import run_bass_kernel_spmd

    x = np.asarray(x)
    seq_len = np.asarray(seq_len)
    W = np.asarray(W)
    b = np.asarray(b)
    _cached.pop("wx", None)  # W may differ between calls
    _cached.pop("wh", None)
    _cached.pop("bT", None)

    # The recurrence only matters up to max(seq_len): every row's output is
    # gathered at hist[seq_len-1], so steps beyond the batch max are dead work.
    n_steps = max(1, int(np.asarray(seq_len).max()))
    if _cached.get("nc_steps") != n_steps:
        _cached["nc"] = _build_program(n_steps=n_steps)
        _cached["nc_steps"] = n_steps
    nc = _cached["nc"]

    in_maps = [_host_prep(x, seq_len, W, b, core) for core in range(NCORES)]
    res = run_bass_kernel_spmd(nc, in_maps, list(range(NCORES)))

    out = np.zeros((B, H), np.float32)
    for core in range(NCORES):
        yk = res.results[core]["y"].reshape(128, 8, 8)   # [p][ub][b]
        out[BPC * core:BPC * (core + 1)] = yk.transpose(2, 1, 0).reshape(BPC, H)
    return out



# revision 5
# speedup vs baseline: 2.0203x; 2.0203x over previous
"""Trainium2 Bass kernel for dynamic-RNN (LSTM, return-last) B=64 T=256 D=512 H=1024.

Strategy: data-parallel over batch across 8 NeuronCores (8 rows per core), no
inter-core communication.  Per core, everything runs in a transposed "fat"
layout with gate units on the partition axis:

  phase 1: zxT[m][p][t*8+b] = (x @ Wx + b).T for the core's 8 batch rows
           (fp16 matmuls, fp32 psum), written to DRAM.
  phase 2: 256 sequential LSTM steps.  Per step, the recurrent matmul
           zh.T = Wh.T @ h.T runs as 32 gate-chunks x 8 K-chunks of
           [128,128]x[128,8] fp16 matmuls (weights stationary), rotated
           over 4 PSUM banks (double-buffered by step parity).  Elementwise
           gates/state run on ACT/DVE in fp32 on [128,64]/[128,192] tiles.
           h is stored fp16 and dumped to a DRAM history each step.
  phase 3: the recurrence runs unmasked; dynamic_rnn's copy-through
           semantics are recovered by gathering hist[seq_len[b]-1] per row
           with register-offset DMAs.

Gate order is host-reordered to [j,i,f,o] so tanh/sigmoid each apply to one
contiguous block; FORGET_BIAS is folded into b on the host.
"""

import numpy as np

B, T, D, H = 64, 256, 512, 1024
NCORES = 8
BPC = B // NCORES              # batch rows per core
GATEMAP_NEW2OLD = [1, 0, 2, 3]  # new order [j,i,f,o] -> old (i,j,f,o) indices

_cached = {}


def _colmap():
    m = np.empty(4 * H, np.int64)
    for g in range(4):
        m[g * H:(g + 1) * H] = GATEMAP_NEW2OLD[g] * H + np.arange(H)
    return m


def _build_program(n_steps=T):
    import concourse.bass as bass
    import concourse.mybir as mybir

    f16 = mybir.dt.float16
    f32 = mybir.dt.float32
    i32 = mybir.dt.int32
    AF = mybir.ActivationFunctionType

    nc = bass.Bass()

    xT_in = nc.dram_tensor("xT", [128, 8192], f16, kind="ExternalInput")
    wx_in = nc.dram_tensor("wx", [128, 16384], f16, kind="ExternalInput")
    wh_in = nc.dram_tensor("wh", [128, 32768], f16, kind="ExternalInput")
    bT_in = nc.dram_tensor("bT", [128, 32], f32, kind="ExternalInput")
    seq_in = nc.dram_tensor("seq", [1, 8], i32, kind="ExternalInput")
    y_out = nc.dram_tensor("y", [128, 64], f32, kind="ExternalOutput")

    zx_dram = nc.dram_tensor("zx_dram", [32, 128, 2048], f32)
    hist = nc.dram_tensor("hist", [n_steps, 128, 64], f16)

    import contextlib
    stack = contextlib.ExitStack()

    sb = lambda name, shape, dt: stack.enter_context(nc.sbuf_tensor(name, shape, dt))
    sem = lambda name: stack.enter_context(nc.semaphore(name))
    xT_sb = sb("xT_sb", [128, 8192], f16)
    wx_sb = sb("wx_sb", [128, 16384], f16)
    wh_sb = sb("wh_sb", [128, 32768], f16)
    bT_sb = sb("bT_sb", [128, 32], f32)
    zst0 = sb("zst0", [128, 2048], f32)
    zst1 = sb("zst1", [128, 2048], f32)
    zxr = sb("zxr", [128, 4 * 256], f32)
    z_sb = sb("z_sb", [128, 256], f32)
    sz = sb("sz", [128, 256], f32)
    cst = sb("cst", [128, 64], f32)
    t1_sb = sb("t1_sb", [128, 64], f32)
    t2_sb = sb("t2_sb", [128, 64], f32)
    th_sb = sb("th_sb", [128, 64], f32)
    h0 = sb("h0", [128, 64], f16)
    h1 = sb("h1", [128, 64], f16)
    hout16 = sb("hout16", [128, 64], f16)
    hy = sb("hy", [128, 64], f32)
    seq_sb = sb("seq_sb", [1, 8], i32)
    spacer = sb("spacer", [128, 1], f32)
    s_in = sem("s_in"); s_p1mm = sem("s_p1mm"); s_p1e = sem("s_p1e")
    s_p1d = sem("s_p1d"); s_pe = sem("s_pe"); s_zx = sem("s_zx")
    s_zxc = sem("s_zxc"); s_za = sem("s_za"); s_act1 = sem("s_act1"); s_dvec = sem("s_dvec")
    s_act2 = sem("s_act2"); s_h = sem("s_h"); s_hist = sem("s_hist")
    s_go = sem("s_go"); s_hy = sem("s_hy"); s_yo = sem("s_yo")
    with nc.Block() as block:
        pss = [stack.enter_context(nc.psum_tensor(f"ps{i}", [128, 512], f32))
               for i in range(8)]
        zst = [zst0, zst1]
        hbuf = [h0, h1]

        # 3D strided views for the canonical z layout: col = 8*(4s+bank)+b
        def bank_view(ap2d, bank):
            v = ap2d.rearrange("p (s k b) -> p s k b", k=4, b=8)
            return v[:, :, bank, :]

        # ---------------- SYNC (SP): loads, zx store/prefetch, hist dump ----
        @block.sync
        def _(sp):
            sp.dma_start(out=xT_sb[:], in_=xT_in[:]).then_inc(s_in, 16)
            sp.dma_start(out=wx_sb[:], in_=wx_in[:]).then_inc(s_in, 16)
            sp.dma_start(out=wh_sb[:], in_=wh_in[:]).then_inc(s_in, 16)
            sp.dma_start(out=bT_sb[:], in_=bT_in[:]).then_inc(s_in, 16)
            # phase 1: store zx stage slabs
            for m in range(32):
                sp.wait_ge(s_p1e, 4 * m + 4)
                sp.dma_start(out=zx_dram[m], in_=zst[m % 2][:]).then_inc(s_p1d, 16)
            # phase 2 prefetch + hist
            zx_src = zx_dram.ap().rearrange("m p q -> p m q")
            for t in range(min(4, n_steps)):
                sp.wait_ge(s_p1d, 512)
                sp.dma_start(
                    out=zxr.ap().rearrange("p (r m b) -> p r m b", m=32, b=8)[:, t % 4],
                    in_=zx_src[:, :, 8 * t:8 * t + 8],
                ).then_inc(s_zx, 16)
            for t in range(n_steps):
                sp.wait_ge(s_h, 2 * (t + 2))
                sp.dma_start(out=hist[t], in_=hbuf[t % 2][:]).then_inc(s_hist, 16)
                if t + 4 < n_steps:
                    sp.wait_ge(s_za, 2 * (t + 1))
                    sp.dma_start(
                        out=zxr.ap().rearrange("p (r m b) -> p r m b", m=32, b=8)[:, (t + 4) % 4],
                        in_=zx_src[:, :, 8 * (t + 4):8 * (t + 4) + 8],
                    ).then_inc(s_zx, 16)

        # ---------------- TENSOR (PE): phase 1 + phase 2 matmuls ------------
        @block.tensor
        def _(te):
            te.wait_ge(s_in, 32)  # xT + wx loaded
            for m in range(32):
                for n in range(4):
                    if m >= 2:
                        te.wait_ge(s_p1e, 4 * (m - 2) + n + 1)
                    bank = (m % 2) * 4 + n
                    for k in range(4):
                        te.matmul(
                            pss[bank][:, 0:512],
                            wx_sb[:, (m * 4 + k) * 128:(m * 4 + k) * 128 + 128],
                            xT_sb[:, k * 2048 + n * 512:k * 2048 + n * 512 + 512],
                            start=(k == 0), stop=(k == 3),
                        ).then_maybe_inc((s_p1mm, 1) if k == 3 else None)
            te.wait_ge(s_in, 48)   # wh loaded
            te.wait_ge(s_p1e, 128)  # all phase-1 evacs done before reusing banks
            for t in range(n_steps):
                # A-pass (K-chunks 0-3) needs only half-0 of h(t-1); runs while
                # DVE/ACT still compute half-1. Partials go to a SEPARATE closed
                # PSUM region (cols 32:64) summed with the B partials by DVE.
                te.wait_ge(s_h, 2 * (t + 1) - 1)
                if t >= 1:
                    te.wait_ge(s_za, 2 * t)  # zadds of step t-1 done (all banks)
                for half in range(2):
                    bs = 4 * half
                    for uo in range(4 * half, 4 * half + 4):
                        for g in range(4):
                            ci = g * 8 + uo
                            for ub in range(4):
                                w_off = (ci * 8 + ub) * 128
                                te.matmul(
                                    pss[bs + g][:, 32 + 8 * (uo % 4):32 + 8 * (uo % 4) + 8],
                                    wh_sb[:, w_off:w_off + 128],
                                    hbuf[(t + 1) % 2][:, 8 * ub:8 * ub + 8],
                                    start=(ub == 0), stop=(ub == 3),
                                )
                te.wait_ge(s_h, 2 * (t + 1))   # half-1 of h(t-1)
                for half in range(2):
                    bs = 4 * half
                    for uo in range(4 * half, 4 * half + 4):
                        for g in range(4):
                            ci = g * 8 + uo
                            for ub in range(4, 8):
                                w_off = (ci * 8 + ub) * 128
                                te.matmul(
                                    pss[bs + g][:, 8 * (uo % 4):8 * (uo % 4) + 8],
                                    wh_sb[:, w_off:w_off + 128],
                                    hbuf[(t + 1) % 2][:, 8 * ub:8 * ub + 8],
                                    start=(ub == 4), stop=(ub == 7),
                                ).then_maybe_inc(
                                    (s_pe, 1) if (g == 3 and ub == 7 and uo % 4 == 3) else None)

        # ---------------- VECTOR (DVE): evacs, z-adds, state ----------------
        @block.vector
        def _(v):
            v.wait_ge(s_in, 64)
            for m in range(32):
                for n in range(4):
                    v.wait_ge(s_p1mm, 4 * m + n + 1)
                    if m >= 2 and n == 0:
                        v.wait_ge(s_p1d, 16 * (m - 1))
                    bank = (m % 2) * 4 + n
                    v.tensor_scalar_add(
                        zst[m % 2][:, 512 * n:512 * n + 512],
                        pss[bank][:, 0:512],
                        bT_sb[:, m:m + 1],
                    ).then_inc(s_p1e, 1)
            # init state
            v.memset(cst[:], 0.0)
            v.memset(hbuf[1][:], 0.0).then_inc(s_h, 2)
            for t in range(n_steps):
                zo = 256 * (t % 4)

                def zadd(H, _t=t, _zo=zo):
                    v.wait_ge(s_pe, 2 * _t + H + 1)
                    if H == 0:
                        v.wait_ge(s_zx, 16 * (_t + 1))
                    for g in range(4):
                        col = g * 64 + 32 * H
                        v.tensor_add(
                            z_sb[:, col:col + 32],
                            pss[4 * H + g][:, 0:32],
                            zxr[:, _zo + col:_zo + col + 32],
                        )
                    for g in range(4):
                        col = g * 64 + 32 * H
                        v.tensor_add(
                            z_sb[:, col:col + 32],
                            z_sb[:, col:col + 32],
                            pss[4 * H + g][:, 32:64],
                        ).then_maybe_inc((s_za, 1) if g == 3 else None)

                def stateAB(H, _t=t):
                    v.wait_ge(s_act1, 2 * _t + H + 1)
                    cc = slice(32 * H, 32 * H + 32)
                    v.tensor_mul(t2_sb[:, cc], cst[:, cc], sz[:, 128 + 32 * H:128 + 32 * H + 32])
                    v.tensor_mul(t1_sb[:, cc], sz[:, 64 + 32 * H:64 + 32 * H + 32], sz[:, 32 * H:32 * H + 32])
                    v.tensor_copy(spacer[:], t2_sb[:, 32 * H:32 * H + 1])
                    v.tensor_add(cst[:, cc], t1_sb[:, cc], t2_sb[:, cc]).then_inc(s_dvec, 1)

                def hmul(H, _t=t):
                    v.wait_ge(s_act2, 2 * _t + H + 1)
                    if H == 0 and _t >= 2:
                        v.wait_ge(s_hist, 16 * (_t - 1))
                    cc = slice(32 * H, 32 * H + 32)
                    v.tensor_mul(hbuf[_t % 2][:, cc], th_sb[:, cc],
                                 sz[:, 192 + 32 * H:192 + 32 * H + 32]).then_inc(s_h, 1)

                zadd(0)
                stateAB(0)
                zadd(1)
                hmul(0)
                stateAB(1)
                hmul(1)
            # final output convert
            v.wait_ge(s_go, 144)
            v.tensor_copy(hy[:], hout16[:]).then_inc(s_hy, 1)

        # ---------------- SCALAR (ACT): gate nonlinearities -----------------
        @block.scalar
        def _(sc):
            for t in range(n_steps):
                for H in range(2):
                    sc.wait_ge(s_za, 2 * t + H + 1)
                    col = 64 + 32 * H
                    sview = bass.AP(sz, col, [[256, 128], [64, 3], [1, 32]])
                    zview = bass.AP(z_sb, col, [[256, 128], [64, 3], [1, 32]])
                    sc.activation(sview, zview, AF.Sigmoid)
                    sc.activation(sz[:, 32 * H:32 * H + 32], z_sb[:, 32 * H:32 * H + 32],
                                  AF.Tanh).then_inc(s_act1, 1)
                    sc.wait_ge(s_dvec, 2 * t + H + 1)
                    sc.activation(th_sb[:, 32 * H:32 * H + 32], cst[:, 32 * H:32 * H + 32],
                                  AF.Tanh).then_inc(s_act2, 1)

        # ---------------- GPSIMD: final gather ------------------------------
        @block.gpsimd
        def _(g):
            g.dma_start(out=seq_sb[:], in_=seq_in[:]).then_inc(s_go, 16)
            g.wait_ge(s_go, 16)
            g.wait_ge(s_hist, 16 * n_steps)
            ctx_nc = nc.allow_non_contiguous_dma(reason="8 tiny strided gather DMAs at kernel end")
            ctx_nc.__enter__()
            for b_i in range(8):
                tr = g.alloc_register(f"t{b_i}")
                g.reg_load(tr, seq_sb[0:1, b_i:b_i + 1])
                g.reg_sub(tr, tr, 1)
                g.reg_mul(tr, tr, 8192)
                g.reg_add(tr, tr, b_i)
                g.dma_start(
                    out=bass.AP(hout16, b_i, [[64, 128], [8, 8]]),
                    in_=bass.AP(hist, tr, [[64, 128], [8, 8]]),
                ).then_inc(s_go, 16)
            ctx_nc.__exit__(None, None, None)
            g.wait_ge(s_hy, 1)
            g.dma_start(out=y_out[:], in_=hy[:]).then_inc(s_yo, 16)
            g.wait_ge(s_yo, 16)

    return nc


def _host_prep(x, seq_len, W, b, core):
    rows = slice(BPC * core, BPC * (core + 1))
    cm = _cached.setdefault("colmap", _colmap())

    xk = np.asarray(x[rows], np.float16)                 # [8,256,512]
    xT = xk.transpose(2, 1, 0).reshape(4, 128, 2048)
    xT = np.ascontiguousarray(xT.transpose(1, 0, 2)).reshape(128, 8192)

    if "wx" not in _cached:
        W16 = np.asarray(W, np.float16)
        wx_full = W16[:512][:, cm]
        wh_full = W16[512:][:, cm]
        _cached["wx"] = np.ascontiguousarray(
            wx_full.reshape(4, 128, 32, 128).transpose(1, 2, 0, 3)).reshape(128, 16384)
        _cached["wh"] = np.ascontiguousarray(
            wh_full.reshape(8, 128, 32, 128).transpose(1, 2, 0, 3)).reshape(128, 32768)
        bp = np.asarray(b, np.float32)[cm].copy()
        bp[2048:3072] += 1.0
        _cached["bT"] = np.ascontiguousarray(bp.reshape(32, 128).T)

    seq = np.asarray(seq_len[rows], np.int32).reshape(1, 8)
    return {
        "xT": xT,
        "wx": _cached["wx"],
        "wh": _cached["wh"],
        "bT": _cached["bT"],
        "seq": seq,
    }


def kernel(x, seq_len, W, b):
    from concourse.bass_utils import run_bass_kernel_spmd

    x = np.asarray(x)
    seq_len = np.asarray(seq_len)
    W = np.asarray(W)
    b = np.asarray(b)
    _cached.pop("wx", None)  # W may differ between calls
    _cached.pop("wh", None)
    _cached.pop("bT", None)

    # The recurrence only matters up to max(seq_len): every row's output is
    # gathered at hist[seq_len-1], so steps beyond the batch max are dead work.
    n_steps = max(1, int(np.asarray(seq_len).max()))
    if _cached.get("nc_steps") != n_steps:
        _cached["nc"] = _build_program(n_steps=n_steps)
        _cached["nc_steps"] = n_steps
    nc = _cached["nc"]

    in_maps = [_host_prep(x, seq_len, W, b, core) for core in range(NCORES)]
    res = run_bass_kernel_spmd(nc, in_maps, list(range(NCORES)))

    out = np.zeros((B, H), np.float32)
    for core in range(NCORES):
        yk = res.results[core]["y"].reshape(128, 8, 8)   # [p][ub][b]
        out[BPC * core:BPC * (core + 1)] = yk.transpose(2, 1, 0).reshape(BPC, H)
    return out



# revision 9
# speedup vs baseline: 6.3496x; 3.1429x over previous
"""Trainium2 Bass kernel for dynamic-RNN (LSTM, return-last) B=64 T=256 D=512 H=1024.

Strategy: data-parallel over batch across 8 NeuronCores (8 rows per core), no
inter-core communication.  Per core, everything runs in a transposed "fat"
layout with gate units on the partition axis:

  phase 1: zxT[m][p][t*8+b] = (x @ Wx + b).T for the core's 8 batch rows
           (fp16 matmuls, fp32 psum), written to DRAM.
  phase 2: 256 sequential LSTM steps.  Per step, the recurrent matmul
           zh.T = Wh.T @ h.T runs as 32 gate-chunks x 8 K-chunks of
           [128,128]x[128,8] fp16 matmuls (weights stationary), rotated
           over 4 PSUM banks (double-buffered by step parity).  Elementwise
           gates/state run on ACT/DVE in fp32 on [128,64]/[128,192] tiles.
           h is stored fp16 and dumped to a DRAM history each step.
  phase 3: the recurrence runs unmasked; dynamic_rnn's copy-through
           semantics are recovered by gathering hist[seq_len[b]-1] per row
           with register-offset DMAs.

Gate order is host-reordered to [j,i,f,o] so tanh/sigmoid each apply to one
contiguous block; FORGET_BIAS is folded into b on the host.
"""

import numpy as np

B, T, D, H = 64, 256, 512, 1024
NCORES = 8
BPC = B // NCORES              # batch rows per core
GATEMAP_NEW2OLD = [1, 0, 2, 3]  # new order [j,i,f,o] -> old (i,j,f,o) indices

_cached = {}


def _colmap():
    m = np.empty(4 * H, np.int64)
    for g in range(4):
        m[g * H:(g + 1) * H] = GATEMAP_NEW2OLD[g] * H + np.arange(H)
    return m


def _build_program(n_steps=T):
    import concourse.bass as bass
    import concourse.mybir as mybir

    f16 = mybir.dt.float16
    f32 = mybir.dt.float32
    i32 = mybir.dt.int32
    AF = mybir.ActivationFunctionType

    nc = bass.Bass()

    xT_in = nc.dram_tensor("xT", [128, 8192], f16, kind="ExternalInput")
    wx_in = nc.dram_tensor("wx", [128, 16384], f16, kind="ExternalInput")
    wh_in = nc.dram_tensor("wh", [128, 32768], f16, kind="ExternalInput")
    bT_in = nc.dram_tensor("bT", [128, 32], f32, kind="ExternalInput")
    seq_in = nc.dram_tensor("seq", [1, 8], i32, kind="ExternalInput")
    y_out = nc.dram_tensor("y", [128, 64], f32, kind="ExternalOutput")

    zx_dram = nc.dram_tensor("zx_dram", [32, 128, 2048], f32)
    hist = nc.dram_tensor("hist", [n_steps, 128, 64], f16)

    import contextlib
    stack = contextlib.ExitStack()

    sb = lambda name, shape, dt: stack.enter_context(nc.sbuf_tensor(name, shape, dt))
    sem = lambda name: stack.enter_context(nc.semaphore(name))
    xT_sb = sb("xT_sb", [128, 8192], f16)
    wx_sb = sb("wx_sb", [128, 16384], f16)
    wh_sb = sb("wh_sb", [128, 32768], f16)
    bT_sb = sb("bT_sb", [128, 32], f32)
    zst0 = sb("zst0", [128, 2048], f32)
    zst1 = sb("zst1", [128, 2048], f32)
    zxr = sb("zxr", [128, 4 * 256], f32)
    z_sb = sb("z_sb", [128, 256], f32)
    sz = sb("sz", [128, 256], f32)
    cst = sb("cst", [128, 64], f32)
    t1_sb = sb("t1_sb", [128, 64], f32)
    t2_sb = sb("t2_sb", [128, 64], f32)
    th_sb = sb("th_sb", [128, 64], f32)
    h0 = sb("h0", [128, 64], f16)
    h1 = sb("h1", [128, 64], f16)
    hout16 = sb("hout16", [128, 64], f16)
    hy = sb("hy", [128, 64], f32)
    seq_sb = sb("seq_sb", [1, 8], i32)
    spacer = sb("spacer", [128, 1], f32)
    s_in = sem("s_in"); s_p1mm = sem("s_p1mm"); s_p1e = sem("s_p1e")
    s_p1d = sem("s_p1d"); s_pe = sem("s_pe"); s_zx = sem("s_zx")
    s_zxc = sem("s_zxc"); s_za = sem("s_za"); s_act1 = sem("s_act1"); s_dvec = sem("s_dvec")
    s_act2 = sem("s_act2"); s_h = sem("s_h"); s_hist = sem("s_hist")
    s_go = sem("s_go"); s_hy = sem("s_hy"); s_yo = sem("s_yo")
    with nc.Block() as block:
        pss = [stack.enter_context(nc.psum_tensor(f"ps{i}", [128, 512], f32))
               for i in range(8)]
        zst = [zst0, zst1]
        hbuf = [h0, h1]

        # 3D strided views for the canonical z layout: col = 8*(4s+bank)+b
        def bank_view(ap2d, bank):
            v = ap2d.rearrange("p (s k b) -> p s k b", k=4, b=8)
            return v[:, :, bank, :]

        # ---------------- SYNC (SP): loads, zx store/prefetch, hist dump ----
        @block.sync
        def _(sp):
            sp.dma_start(out=xT_sb[:], in_=xT_in[:]).then_inc(s_in, 16)
            sp.dma_start(out=wx_sb[:], in_=wx_in[:]).then_inc(s_in, 16)
            sp.dma_start(out=wh_sb[:], in_=wh_in[:]).then_inc(s_in, 16)
            sp.dma_start(out=bT_sb[:], in_=bT_in[:]).then_inc(s_in, 16)
            # phase 1: store zx stage slabs
            for m in range(32):
                sp.wait_ge(s_p1e, 4 * m + 4)
                sp.dma_start(out=zx_dram[m], in_=zst[m % 2][:]).then_inc(s_p1d, 16)
            # phase 2 prefetch + hist
            zx_src = zx_dram.ap().rearrange("m p q -> p m q")
            for t in range(min(4, n_steps)):
                sp.wait_ge(s_p1d, 512)
                sp.dma_start(
                    out=zxr.ap().rearrange("p (r m b) -> p r m b", m=32, b=8)[:, t % 4],
                    in_=zx_src[:, :, 8 * t:8 * t + 8],
                ).then_inc(s_zx, 16)
            for t in range(n_steps):
                sp.wait_ge(s_h, 2 * (t + 2))
                sp.dma_start(out=hist[t], in_=hbuf[t % 2][:]).then_inc(s_hist, 16)
                if t + 4 < n_steps:
                    sp.wait_ge(s_za, 6 * (t + 1))
                    sp.dma_start(
                        out=zxr.ap().rearrange("p (r m b) -> p r m b", m=32, b=8)[:, (t + 4) % 4],
                        in_=zx_src[:, :, 8 * (t + 4):8 * (t + 4) + 8],
                    ).then_inc(s_zx, 16)

        # ---------------- TENSOR (PE): phase 1 + phase 2 matmuls ------------
        @block.tensor
        def _(te):
            te.wait_ge(s_in, 32)  # xT + wx loaded
            for m in range(32):
                for n in range(4):
                    if m >= 2:
                        te.wait_ge(s_p1e, 4 * (m - 2) + n + 1)
                    bank = (m % 2) * 4 + n
                    for k in range(4):
                        te.matmul(
                            pss[bank][:, 0:512],
                            wx_sb[:, (m * 4 + k) * 128:(m * 4 + k) * 128 + 128],
                            xT_sb[:, k * 2048 + n * 512:k * 2048 + n * 512 + 512],
                            start=(k == 0), stop=(k == 3),
                        ).then_maybe_inc((s_p1mm, 1) if k == 3 else None)
            te.wait_ge(s_in, 48)   # wh loaded
            te.wait_ge(s_p1e, 128)  # all phase-1 evacs done before reusing banks
            for t in range(n_steps):
                # A-pass (K-chunks 0-3) needs only half-0 of h(t-1); runs while
                # DVE/ACT still compute half-1. Partials go to a SEPARATE closed
                # PSUM region (cols 32:64) summed with the B partials by DVE.
                te.wait_ge(s_h, 2 * (t + 1) - 1)
                if t >= 1:
                    te.wait_ge(s_za, 6 * t)  # zadds of step t-1 done (all banks)
                for half in range(2):
                    bs = 4 * half
                    for uo in range(4 * half, 4 * half + 4):
                        for g in range(4):
                            ci = g * 8 + uo
                            for ub in range(4):
                                w_off = (ci * 8 + ub) * 128
                                te.matmul(
                                    pss[bs + g][:, 32 + 8 * (uo % 4):32 + 8 * (uo % 4) + 8],
                                    wh_sb[:, w_off:w_off + 128],
                                    hbuf[(t + 1) % 2][:, 8 * ub:8 * ub + 8],
                                    start=(ub == 0), stop=(ub == 3),
                                )
                te.wait_ge(s_h, 2 * (t + 1))   # half-1 of h(t-1)
                for half in range(2):
                    bs = 4 * half
                    for uo in range(4 * half, 4 * half + 4):
                        for g in range(4):
                            ci = g * 8 + uo
                            for ub in range(4, 8):
                                w_off = (ci * 8 + ub) * 128
                                te.matmul(
                                    pss[bs + g][:, 8 * (uo % 4):8 * (uo % 4) + 8],
                                    wh_sb[:, w_off:w_off + 128],
                                    hbuf[(t + 1) % 2][:, 8 * ub:8 * ub + 8],
                                    start=(ub == 4), stop=(ub == 7),
                                ).then_maybe_inc(
                                    (s_pe, 1) if (g == 3 and ub == 7 and uo % 4 == 3) else None)

        # ---------------- VECTOR (DVE): evacs, z-adds, state ----------------
        @block.vector
        def _(v):
            v.wait_ge(s_in, 64)
            for m in range(32):
                for n in range(4):
                    v.wait_ge(s_p1mm, 4 * m + n + 1)
                    if m >= 2 and n == 0:
                        v.wait_ge(s_p1d, 16 * (m - 1))
                    bank = (m % 2) * 4 + n
                    v.tensor_scalar_add(
                        zst[m % 2][:, 512 * n:512 * n + 512],
                        pss[bank][:, 0:512],
                        bT_sb[:, m:m + 1],
                    ).then_inc(s_p1e, 1)
            # init state
            v.memset(cst[:], 0.0)
            v.memset(hbuf[1][:], 0.0).then_inc(s_h, 2)
            for t in range(n_steps):
                zo = 256 * (t % 4)

                def zadd(H, _t=t, _zo=zo):
                    # gate order (i,f) -> j -> o so ACT can start sigmoid(i,f)
                    # and tanh(j) while the remaining adds run; s_za +3/half
                    v.wait_ge(s_pe, 2 * _t + H + 1)
                    if H == 0:
                        v.wait_ge(s_zx, 16 * (_t + 1))

                    def a1(g):
                        col = g * 64 + 32 * H
                        return v.tensor_add(
                            z_sb[:, col:col + 32],
                            pss[4 * H + g][:, 0:32],
                            zxr[:, _zo + col:_zo + col + 32],
                        )

                    def a2(g):
                        col = g * 64 + 32 * H
                        return v.tensor_add(
                            z_sb[:, col:col + 32],
                            z_sb[:, col:col + 32],
                            pss[4 * H + g][:, 32:64],
                        )

                    a1(1); a1(2); a2(1)
                    a2(2).then_inc(s_za, 1)   # sigmoid(i,f) ready
                    a1(0); a1(3)
                    a2(0).then_inc(s_za, 1)   # tanh(j) ready
                    a2(3).then_inc(s_za, 1)   # sigmoid(o) ready

                def stateAB(H, _t=t):
                    v.wait_ge(s_act1, 2 * _t + H + 1)
                    cc = slice(32 * H, 32 * H + 32)
                    v.tensor_mul(t2_sb[:, cc], cst[:, cc], sz[:, 128 + 32 * H:128 + 32 * H + 32])
                    v.tensor_mul(t1_sb[:, cc], sz[:, 64 + 32 * H:64 + 32 * H + 32], sz[:, 32 * H:32 * H + 32])
                    v.tensor_copy(spacer[:], t2_sb[:, 32 * H:32 * H + 1])
                    v.tensor_add(cst[:, cc], t1_sb[:, cc], t2_sb[:, cc]).then_inc(s_dvec, 1)

                def hmul(H, _t=t):
                    v.wait_ge(s_act2, 2 * _t + H + 1)
                    if H == 0 and _t >= 2:
                        v.wait_ge(s_hist, 16 * (_t - 1))
                    cc = slice(32 * H, 32 * H + 32)
                    v.tensor_mul(hbuf[_t % 2][:, cc], th_sb[:, cc],
                                 sz[:, 192 + 32 * H:192 + 32 * H + 32]).then_inc(s_h, 1)

                zadd(0)
                stateAB(0)
                zadd(1)
                hmul(0)
                stateAB(1)
                hmul(1)
            # final output convert
            v.wait_ge(s_go, 144)
            v.tensor_copy(hy[:], hout16[:]).then_inc(s_hy, 1)

        # ---------------- SCALAR (ACT): gate nonlinearities -----------------
        @block.scalar
        def _(sc):
            for t in range(n_steps):
                for H in range(2):
                    base = 6 * t + 3 * H
                    col = 64 + 32 * H
                    sc.wait_ge(s_za, base + 1)
                    sview = bass.AP(sz, col, [[256, 128], [64, 2], [1, 32]])
                    zview = bass.AP(z_sb, col, [[256, 128], [64, 2], [1, 32]])
                    sc.activation(sview, zview, AF.Sigmoid)   # gates i, f
                    sc.wait_ge(s_za, base + 2)
                    sc.activation(sz[:, 32 * H:32 * H + 32], z_sb[:, 32 * H:32 * H + 32],
                                  AF.Tanh).then_inc(s_act1, 1)
                    sc.wait_ge(s_za, base + 3)
                    sc.activation(sz[:, 192 + 32 * H:192 + 32 * H + 32],
                                  z_sb[:, 192 + 32 * H:192 + 32 * H + 32],
                                  AF.Sigmoid)                 # gate o
                    sc.wait_ge(s_dvec, 2 * t + H + 1)
                    sc.activation(th_sb[:, 32 * H:32 * H + 32], cst[:, 32 * H:32 * H + 32],
                                  AF.Tanh).then_inc(s_act2, 1)

        # ---------------- GPSIMD: final gather ------------------------------
        @block.gpsimd
        def _(g):
            g.dma_start(out=seq_sb[:], in_=seq_in[:]).then_inc(s_go, 16)
            g.wait_ge(s_go, 16)
            g.wait_ge(s_hist, 16 * n_steps)
            ctx_nc = nc.allow_non_contiguous_dma(reason="8 tiny strided gather DMAs at kernel end")
            ctx_nc.__enter__()
            for b_i in range(8):
                tr = g.alloc_register(f"t{b_i}")
                g.reg_load(tr, seq_sb[0:1, b_i:b_i + 1])
                g.reg_sub(tr, tr, 1)
                g.reg_mul(tr, tr, 8192)
                g.reg_add(tr, tr, b_i)
                g.dma_start(
                    out=bass.AP(hout16, b_i, [[64, 128], [8, 8]]),
                    in_=bass.AP(hist, tr, [[64, 128], [8, 8]]),
                ).then_inc(s_go, 16)
            ctx_nc.__exit__(None, None, None)
            g.wait_ge(s_hy, 1)
            g.dma_start(out=y_out[:], in_=hy[:]).then_inc(s_yo, 16)
            g.wait_ge(s_yo, 16)

    return nc


def _host_prep(x, seq_len, W, b, core):
    rows = slice(BPC * core, BPC * (core + 1))
    cm = _cached.setdefault("colmap", _colmap())

    xk = np.asarray(x[rows], np.float16)                 # [8,256,512]
    xT = xk.transpose(2, 1, 0).reshape(4, 128, 2048)
    xT = np.ascontiguousarray(xT.transpose(1, 0, 2)).reshape(128, 8192)

    if "wx" not in _cached:
        W16 = np.asarray(W, np.float16)
        wx_full = W16[:512][:, cm]
        wh_full = W16[512:][:, cm]
        _cached["wx"] = np.ascontiguousarray(
            wx_full.reshape(4, 128, 32, 128).transpose(1, 2, 0, 3)).reshape(128, 16384)
        _cached["wh"] = np.ascontiguousarray(
            wh_full.reshape(8, 128, 32, 128).transpose(1, 2, 0, 3)).reshape(128, 32768)
        bp = np.asarray(b, np.float32)[cm].copy()
        bp[2048:3072] += 1.0
        _cached["bT"] = np.ascontiguousarray(bp.reshape(32, 128).T)

    seq = np.asarray(seq_len[rows], np.int32).reshape(1, 8)
    return {
        "xT": xT,
        "wx": _cached["wx"],
        "wh": _cached["wh"],
        "bT": _cached["bT"],
        "seq": seq,
    }


def kernel(x, seq_len, W, b):
    from concourse.bass_utils import run_bass_kernel_spmd

    x = np.asarray(x)
    seq_len = np.asarray(seq_len)
    W = np.asarray(W)
    b = np.asarray(b)
    _cached.pop("wx", None)  # W may differ between calls
    _cached.pop("wh", None)
    _cached.pop("bT", None)

    # The recurrence only matters up to max(seq_len): every row's output is
    # gathered at hist[seq_len-1], so steps beyond the batch max are dead work.
    n_steps = max(1, int(np.asarray(seq_len).max()))
    if _cached.get("nc_steps") != n_steps:
        _cached["nc"] = _build_program(n_steps=n_steps)
        _cached["nc_steps"] = n_steps
    nc = _cached["nc"]

    in_maps = [_host_prep(x, seq_len, W, b, core) for core in range(NCORES)]
    res = run_bass_kernel_spmd(nc, in_maps, list(range(NCORES)))

    out = np.zeros((B, H), np.float32)
    for core in range(NCORES):
        yk = res.results[core]["y"].reshape(128, 8, 8)   # [p][ub][b]
        out[BPC * core:BPC * (core + 1)] = yk.transpose(2, 1, 0).reshape(BPC, H)
    return out

